# revision 3
# baseline (speedup 1.0000x reference)
"""Trainium2 Bass kernel for nn_CaptioningRNN (attention-LSTM over T=128 steps).

v2: tensor-parallel over the 4H gate dim across 8 cores, with a stacked
batch layout [128 partitions = 2 h-col-halves x 64 batch] so every
elementwise / ScalarE op runs on all 128 partitions, and every matmul is
col-tiled (tile_position) so both 64-partition output halves compute
concurrently.

Per-core layouts (core j owns h-cols [128j, 128j+128)):
  pa   [128, 256] f32   partition 64a+n = batch n, h-col half a; free =
                        4 gates x 64 cols (i|f|o|g)
  c    [128, 64]  f32   cell state, h_bf [128, 64] bf16 hidden
  hpay [128, 64]  bf16  exchange payload: hpay[64a+i, n] = h[n, 128j+64a+i]
  spay [128, 16]  bf16  score partials: cols 0:8 = own l-half (l = 8a+lh),
                        cols 8:16 = other half (uniform convention across
                        cores; downstream ops never need true l order)

Score partials are computed ON THE PE: one col-tiled all-pairs matmul
(hpay^T x scaled-A), then a mask+strided-reduce diag extraction, then a
permutation matmul (swap partition halves) to fill the other l-half.

Exchange: two AllGathers per step (h 16KB, scores 4KB) triggered
back-to-back so they overlap on the collective engine; single recv DMA per
payload on the Sync HWDGE queue.
"""
import numpy as np
import ml_dtypes

import bass_rust
import concourse.bass as bass
import concourse.mybir as mybir
from concourse import tile
from concourse.alu_op_type import AluOpType
from concourse.bass_utils import run_bass_kernel_spmd

BF16 = ml_dtypes.bfloat16
F32 = mybir.dt.float32
BF = mybir.dt.bfloat16
AF = mybir.ActivationFunctionType
AX = mybir.AxisListType

N, T, D, H, L, R = 64, 128, 512, 1024, 16, 8
HS, GS = H // R, 4 * H // R  # 128, 512
LH = L // 2  # 8
SCALE = 1.0 / np.sqrt(H)


def _split_waits(nc, cap=1):
    """Walrus rejects >cap sync waits per instruction; hoist extras onto
    preceding same-engine NOPs."""
    ctr = 0
    for fn in nc.m.functions:
        for bb in fn.blocks:
            out, changed = [], False
            for ins in bb.instructions:
                si = ins.sync_info
                if si is not None and si.on_wait and len(si.on_wait) > cap:
                    waits = list(si.on_wait)
                    extra, keep = waits[:-cap], waits[-cap:]
                    for i in range(0, len(extra), cap):
                        out.append(bass_rust.InstNoOp(
                            name=f"zz_waitsplit_{ctr}", engine=ins.engine,
                            sync_info=bass_rust.SyncInfo(
                                on_wait=extra[i:i + cap], on_update=[])))
                        ctr += 1
                    ins.sync_info = bass_rust.SyncInfo(
                        on_wait=keep, on_update=list(si.on_update or []))
                    changed = True
                out.append(ins)
            if changed:
                bb.instructions = out
    return ctr


def _prep_inputs(x, A, Wx, Wh, Wattn, b):
    x = np.asarray(x, np.float32)
    A_flat = np.asarray(A, np.float32).reshape(N, H, L)
    Wx = np.asarray(Wx, np.float32)
    Wh = np.asarray(Wh, np.float32)
    Wattn = np.asarray(Wattn, np.float32)
    b = np.asarray(b, np.float32)

    h0 = A_flat.mean(axis=2).astype(np.float32)          # (N, H)
    scores0 = (np.einsum('nh,nhl->nl', h0, A_flat) * SCALE)  # (N, L)

    # x transposed per step/k-tile: xT[t, kt, p, n] = x[n, t, 128kt+p]
    xT = np.ascontiguousarray(
        x.transpose(1, 2, 0).reshape(T, 4, 128, N)).astype(BF16)
    # asTf for the Bst precompute (raw A): [p, r, l, n']
    asTf = np.ascontiguousarray(
        A_flat.transpose(1, 2, 0).reshape(8, 128, L, N).transpose(1, 0, 2, 3)
    ).astype(BF16)
    # estack delta mask: dmE[64a+n', lh, n] = delta(n', n)
    eye = np.eye(N, dtype=np.float32)
    dmE = np.broadcast_to(
        np.concatenate([eye, eye], axis=0)[:, None, :], (128, LH, N))
    dmE = np.ascontiguousarray(dmE).astype(BF16)
    # all-pairs diag mask: dm[64a+n, n'*8+lh] = delta(n, n')
    dm = np.concatenate([eye, eye], axis=0)  # (128, 64)
    dm = np.ascontiguousarray(
        np.broadcast_to(dm[:, :, None], (128, N, LH)).reshape(128, N * LH)
    ).astype(BF16)
    # half-swap permutation: psw = eyesw^T @ x -> out[m] = x[m XOR 64]
    eyesw = np.zeros((128, 128), dtype=np.float32)
    for m in range(128):
        eyesw[m ^ 64, m] = 1.0
    eyesw = eyesw.astype(BF16)
    # identities for the two half transposes (both partition halves)
    eyes2 = np.concatenate([np.eye(N), np.eye(N)], axis=0).astype(BF16)

    in_maps = []
    for j in range(R):
        colsA = np.array([g * H + j * HS + i for g in range(4) for i in range(64)])
        colsB = colsA + 64
        cols = np.concatenate([colsA, colsB])  # 512
        # h0T slot layout (AllGather is rank-ordered): h0T[p, k, n] =
        # h0[n, k*128 + p]
        h0T = np.empty((128, R, N), dtype=np.float32)
        for k in range(R):
            h0T[:, k, :] = h0[:, k * HS:(k + 1) * HS].T
        # initial scores in payload convention
        s0 = np.empty((128, L), dtype=np.float32)
        for a in range(2):
            rows = slice(64 * a, 64 * a + 64)
            s0[rows, 0:LH] = scores0[:, 8 * a:8 * a + 8]
            s0[rows, LH:L] = scores0[:, 8 * (1 - a):8 * (1 - a) + 8]
        # c0 stacked
        c0 = np.empty((128, 64), dtype=np.float32)
        for a in range(2):
            c0[64 * a:64 * a + 64, :] = h0[:, j * HS + 64 * a:j * HS + 64 * a + 64]
        # whj[k, p, f]: rows Wh[k*128+p] (rank-ordered slots), gate-layout cols
        whj = Wh.reshape(R, HS, 4 * H)[:, :, cols]
        wxj = Wx.reshape(4, 128, 4 * H)[:, :, cols]
        waj = Wattn.reshape(8, 128, 4 * H)[:, :, cols]
        # asT for spart all-pairs: asT[p, a_l, n'*8+lh] =
        #     SCALE * A[n', 128j+p, 8*a_l+lh]
        asl = A_flat[:, j * HS:(j + 1) * HS, :] * SCALE   # (n', 128, L)
        asT = np.ascontiguousarray(
            asl.transpose(1, 2, 0)                         # (p, L, n')
            .reshape(128, 2, LH, N)                        # (p, a_l, lh, n')
            .transpose(0, 1, 3, 2)                         # (p, a_l, n', lh)
            .reshape(128, 2, N * LH)
        ).astype(BF16)
        in_maps.append({
            "xT": xT,
            "whj": np.ascontiguousarray(whj).astype(BF16),
            "wxj": np.ascontiguousarray(wxj).astype(BF16),
            "waj": np.ascontiguousarray(waj).astype(BF16),
            "brep": np.tile(b[cols], (128, 1)).astype(np.float32),
            "asTf": asTf,
            "asT": asT,
            "dm": dm,
            "dmE": dmE,
            "eyesw": eyesw,
            "eyes2": eyes2,
            "h0T": np.ascontiguousarray(h0T).astype(BF16),
            "s0": s0.astype(np.float32),
            "c0": c0,
        })
    return in_maps


def _build():
    nc = bass.Bass("TRN2", target_bir_lowering=False, debug=False, num_devices=R)
    rg = [list(range(R))]

    xT_d = nc.dram_tensor("xT", [T, 4, 128, N], BF, kind="ExternalInput")
    whj_d = nc.dram_tensor("whj", [R, 128, GS], BF, kind="ExternalInput")
    wxj_d = nc.dram_tensor("wxj", [4, 128, GS], BF, kind="ExternalInput")
    waj_d = nc.dram_tensor("waj", [R, 128, GS], BF, kind="ExternalInput")
    brep_d = nc.dram_tensor("brep", [128, GS], F32, kind="ExternalInput")
    asTf_d = nc.dram_tensor("asTf", [128, 8, L, N], BF, kind="ExternalInput")
    asT_d = nc.dram_tensor("asT", [128, 2, N * LH], BF, kind="ExternalInput")
    dm_d = nc.dram_tensor("dm", [128, N * LH], BF, kind="ExternalInput")
    dmE_d = nc.dram_tensor("dmE", [128, LH, N], BF, kind="ExternalInput")
    eyesw_d = nc.dram_tensor("eyesw", [128, 128], BF, kind="ExternalInput")
    eyes2_d = nc.dram_tensor("eyes2", [128, N], BF, kind="ExternalInput")
    h0T_d = nc.dram_tensor("h0T", [128, R, N], BF, kind="ExternalInput")
    s0_d = nc.dram_tensor("s0", [128, L], F32, kind="ExternalInput")
    c0_d = nc.dram_tensor("c0", [128, 64], F32, kind="ExternalInput")
    out_d = nc.dram_tensor("out", [128, T, 64], BF, kind="ExternalOutput")

    with tile.TileContext(nc) as tc:
        with tc.tile_pool(name="const", bufs=1) as cp, \
             tc.tile_pool(name="state", bufs=1) as st, \
             tc.tile_pool(name="dram", bufs=2, space="DRAM") as dp:

            whj = cp.tile([128, R, GS], BF, name="whj")
            wxj = cp.tile([128, 4, GS], BF, name="wxj")
            brep = cp.tile([128, GS], F32, name="brep")
            asT = cp.tile([128, 2, N * LH], BF, name="asT")
            dm = cp.tile([128, N * LH], BF, name="dm")
            dmE = cp.tile([128, LH, N], BF, name="dmE")
            eyesw = cp.tile([128, 128], BF, name="eyesw")
            eyes2 = cp.tile([128, N], BF, name="eyes2")
            Bst = cp.tile([128, LH, GS], BF, name="Bst")
            nc.sync.dma_start(out=whj[:, :, :], in_=whj_d.rearrange("k p g -> p k g"))
            nc.sync.dma_start(out=wxj[:, :, :], in_=wxj_d.rearrange("k p g -> p k g"))
            nc.sync.dma_start(out=brep[:, :], in_=brep_d[:, :])
            nc.sync.dma_start(out=asT[:, :, :], in_=asT_d[:, :, :])
            nc.sync.dma_start(out=dm[:, :], in_=dm_d[:, :])
            nc.sync.dma_start(out=dmE[:, :, :], in_=dmE_d[:, :, :])
            nc.sync.dma_start(out=eyesw[:, :], in_=eyesw_d[:, :])
            nc.sync.dma_start(out=eyes2[:, :], in_=eyes2_d[:, :])

            c = st.tile([128, 64], F32, name="c")
            nc.sync.dma_start(out=c[:, :], in_=c0_d[:, :])

            # ---- precompute Bst[64a'+n', lh, :] = A_{8a'+lh}[n']^T @ Wattn_j + b
            with tc.tile_pool(name="pre", bufs=1) as pp, \
                 tc.tile_pool(name="ps_b", bufs=2, space="PSUM") as ps_b:
                asTf = pp.tile([128, 8, L, N], BF, name="asTf")
                waj = pp.tile([128, 8, GS], BF, name="waj")
                nc.sync.dma_start(out=asTf[:, :, :, :], in_=asTf_d[:, :, :, :])
                nc.sync.dma_start(out=waj[:, :, :], in_=waj_d.rearrange("k p g -> p k g"))
                for lh in range(LH):
                    pb = ps_b.tile([128, GS], F32, name="pb", tag="pb")
                    for r in range(8):
                        nc.tensor.matmul(pb[0:64, :], asTf[:, r, lh, :],
                                         waj[:, r, :], start=(r == 0),
                                         stop=(r == 7), tile_position=(0, 0))
                        nc.tensor.matmul(pb[64:128, :], asTf[:, r, 8 + lh, :],
                                         waj[:, r, :], start=(r == 0),
                                         stop=(r == 7), tile_position=(0, 64))
                    nc.vector.tensor_add(out=Bst[:, lh, :], in0=pb[:, :],
                                         in1=brep[:, :])

            with tc.tile_pool(name="wk", bufs=2) as wk, \
                 tc.tile_pool(name="ps_a", bufs=2, space="PSUM") as ps_a, \
                 tc.tile_pool(name="ps_t", bufs=2, space="PSUM") as ps_t, \
                 tc.tile_pool(name="ps_s", bufs=2, space="PSUM") as ps_s, \
                 tc.tile_pool(name="ps_w", bufs=2, space="PSUM") as ps_w:

                hkt = wk.tile([128, R, N], BF, name="hkt0", tag="hkt")
                nc.sync.dma_start(out=hkt[:, :, :], in_=h0T_d[:, :, :])
                sc16 = wk.tile([128, L], F32, name="sc160", tag="sc16")
                nc.sync.dma_start(out=sc16[:, :], in_=s0_d[:, :])
                xtile = wk.tile([128, 4, N], BF, name="xt0", tag="xtile")
                nc.scalar.dma_start(out=xtile[:, :, :],
                                    in_=xT_d[0].rearrange("k p n -> p k n"))
                sprecv = None

                for t in range(T):
                    # ---- gate preactivations: x part (prev AG window)
                    pa = ps_a.tile([128, GS // 2], F32, name="pa", tag="pa")
                    for kt in range(4):
                        nc.tensor.matmul(pa[0:64, :], xtile[:, kt, :],
                                         wxj[:, kt, 0:256],
                                         start=(kt == 0), stop=False,
                                         tile_position=(0, 0))
                        nc.tensor.matmul(pa[64:128, :], xtile[:, kt, :],
                                         wxj[:, kt, 256:512],
                                         start=(kt == 0), stop=False,
                                         tile_position=(0, 64))

                    # ---- softmax chain (sc16 = summed scores, payload order)
                    if t > 0:
                        sc16 = wk.tile([128, L], F32, name="sc16", tag="sc16")
                        nc.vector.reduce_sum(
                            out=sc16[:, :],
                            in_=sprecv.rearrange("p r l -> p l r"),
                            axis=AX.X)
                    e = wk.tile([128, L], F32, name="e", tag="e")
                    nc.scalar.activation(e[:, :], sc16[:, :], AF.Exp)
                    se = wk.tile([128, 1], F32, name="se", tag="se")
                    nc.vector.reduce_sum(out=se[:, :], in_=e[:, :], axis=AX.X)
                    rse = wk.tile([128, 1], F32, name="rse", tag="rse")
                    nc.vector.reciprocal(out=rse[:, :], in_=se[:, :])
                    wl = wk.tile([128, LH], BF, name="wl", tag="wl")
                    nc.vector.tensor_scalar(out=wl[:, :], in0=e[:, 0:LH],
                                            scalar1=rse[:, 0:1], scalar2=None,
                                            op0=AluOpType.mult)
                    estack = wk.tile([128, LH, N], BF, name="estack",
                                     tag="estack")
                    nc.vector.tensor_tensor(
                        out=estack[:, :, :], in0=dmE[:, :, :],
                        in1=wl[:, :, None].broadcast_to((128, LH, N)),
                        op=AluOpType.mult)

                    # ---- h part, then attn part
                    for r in range(8):
                        nc.tensor.matmul(pa[0:64, :], hkt[:, r, :],
                                         whj[:, r, 0:256], start=False,
                                         stop=False, tile_position=(0, 0))
                        nc.tensor.matmul(pa[64:128, :], hkt[:, r, :],
                                         whj[:, r, 256:512], start=False,
                                         stop=False, tile_position=(0, 64))
                    for lh in range(LH):
                        nc.tensor.matmul(pa[0:64, :], estack[:, lh, :],
                                         Bst[:, lh, 0:256], start=False,
                                         stop=(lh == LH - 1),
                                         tile_position=(0, 0))
                        nc.tensor.matmul(pa[64:128, :], estack[:, lh, :],
                                         Bst[:, lh, 256:512], start=False,
                                         stop=(lh == LH - 1),
                                         tile_position=(0, 64))

                    # ---- activations + cell update (gates i|f|o|g x 64)
                    th3 = wk.tile([128, 192], F32, name="th3", tag="th3")
                    nc.scalar.activation(th3[:, :], pa[:, 0:192], AF.Tanh,
                                         scale=0.5)
                    gt = wk.tile([128, 64], F32, name="gt", tag="gt")
                    nc.scalar.activation(gt[:, :], pa[:, 192:256], AF.Tanh)
                    sig = wk.tile([128, 192], F32, name="sig", tag="sig")
                    nc.vector.tensor_scalar(out=sig[:, :], in0=th3[:, :],
                                            scalar1=1.0, scalar2=0.5,
                                            op0=AluOpType.add,
                                            op1=AluOpType.mult)
                    t1 = wk.tile([128, 64], F32, name="t1", tag="t1")
                    nc.vector.tensor_mul(out=t1[:, :], in0=sig[:, 0:64],
                                         in1=gt[:, :])
                    nc.vector.tensor_mul(out=c[:, :], in0=sig[:, 64:128],
                                         in1=c[:, :])
                    nc.vector.tensor_add(out=c[:, :], in0=c[:, :], in1=t1[:, :])
                    tanc = wk.tile([128, 64], F32, name="tanc", tag="tanc")
                    nc.scalar.activation(tanc[:, :], c[:, :], AF.Tanh)
                    h_bf = wk.tile([128, 64], BF, name="h_bf", tag="h_bf")
                    nc.vector.tensor_mul(out=h_bf[:, :], in0=sig[:, 128:192],
                                         in1=tanc[:, :])
                    nc.scalar.dma_start(out=out_d[:, t, :], in_=h_bf[:, :])
                    if t == T - 1:
                        break

                    # ---- h payload: two half transposes into one psum tile
                    pt = ps_t.tile([128, N], BF, name="pt", tag="pt")
                    nc.tensor.transpose(pt[0:64, :], h_bf[0:64, :],
                                        eyes2[0:64, :])
                    nc.tensor.transpose(pt[64:128, :], h_bf[64:128, :],
                                        eyes2[64:128, :])
                    hpay = wk.tile([128, N], BF, name="hpay", tag="hpay")
                    nc.vector.tensor_copy(out=hpay[:, :], in_=pt[:, :])
                    sendH = dp.tile([128 * N], BF, name="sendH", tag="sendH")
                    nc.scalar.dma_start(
                        out=sendH[:].rearrange("(p n) -> p n", p=128),
                        in_=hpay[:, :])
                    recvH = dp.tile([R, 128 * N], BF, name="recvH", tag="recvH",
                                    addr_space="Shared")
                    nc.gpsimd.collective_compute(
                        "AllGather", AluOpType.bypass, replica_groups=rg,
                        ins=[sendH[:].opt()], outs=[recvH[:, :].opt()])

                    # ---- score partials on PE: all-pairs + diag + half-swap
                    psA = ps_s.tile([128, N * LH], F32, name="psA", tag="psA")
                    nc.tensor.matmul(psA[0:64, :], hpay[:, :], asT[:, 0, :],
                                     start=True, stop=True,
                                     tile_position=(0, 0))
                    nc.tensor.matmul(psA[64:128, :], hpay[:, :], asT[:, 1, :],
                                     start=True, stop=True,
                                     tile_position=(0, 64))
                    msk = wk.tile([128, N * LH], BF, name="msk", tag="msk")
                    nc.vector.tensor_tensor(out=msk[:, :], in0=psA[:, :],
                                            in1=dm[:, :], op=AluOpType.mult)
                    spay = wk.tile([128, L], BF, name="spay", tag="spay")
                    with nc.allow_low_precision(reason="bf16 score partials"):
                        nc.vector.reduce_sum(
                            out=spay[:, 0:LH],
                            in_=msk[:, :].rearrange("p (np lh) -> p lh np",
                                                    lh=LH),
                            axis=AX.X)
                    psw = ps_w.tile([128, LH], F32, name="psw", tag="psw")
                    nc.tensor.matmul(psw[:, :], eyesw[:, :], spay[:, 0:LH],
                                     start=True, stop=True)
                    nc.vector.tensor_copy(out=spay[:, LH:L], in_=psw[:, :])
                    sendS = dp.tile([128 * L], BF, name="sendS", tag="sendS")
                    nc.scalar.dma_start(
                        out=sendS[:].rearrange("(p l) -> p l", p=128),
                        in_=spay[:, :])
                    recvS = dp.tile([R, 128 * L], BF, name="recvS", tag="recvS",
                                    addr_space="Shared")
                    nc.gpsimd.collective_compute(
                        "AllGather", AluOpType.bypass, replica_groups=rg,
                        ins=[sendS[:].opt()], outs=[recvS[:, :].opt()])

                    # ---- receives (single DMA each) + x prefetch
                    hkt = wk.tile([128, R, N], BF, name="hkt", tag="hkt")
                    nc.sync.dma_start(
                        out=hkt[:, :, :],
                        in_=recvH[:, :].rearrange("r (p n) -> p r n", p=128))
                    sprecv = wk.tile([128, R, L], BF, name="sprecv",
                                     tag="sprecv")
                    nc.sync.dma_start(
                        out=sprecv[:, :, :],
                        in_=recvS[:, :].rearrange("r (p l) -> p r l", p=128))
                    xtile = wk.tile([128, 4, N], BF, name="xt", tag="xtile")
                    nc.scalar.dma_start(
                        out=xtile[:, :, :],
                        in_=xT_d[t + 1].rearrange("k p n -> p k n"))

    _split_waits(nc, cap=1)
    return nc


_NC_CACHE = None


def _assemble(res) -> np.ndarray:
    out = np.zeros((N, T, H), dtype=np.float32)
    for j, r in enumerate(res.results):
        o = np.asarray(r["out"]).astype(np.float32)  # [128, T, 64]
        o = o.reshape(2, 64, T, 64)                  # [a, n, t, i]
        out[:, :, j * HS:j * HS + 64] = o[0]
        out[:, :, j * HS + 64:j * HS + 128] = o[1]
    return out


def kernel(**inputs) -> np.ndarray:
    global _NC_CACHE
    in_maps = _prep_inputs(**inputs)
    if _NC_CACHE is None:
        _NC_CACHE = _build()
    res = run_bass_kernel_spmd(_NC_CACHE, in_maps, core_ids=list(range(R)))
    return _assemble(res)


# revision 5
# speedup vs baseline: 1.0768x; 1.0768x over previous
"""Trainium2 Bass kernel for nn_CaptioningRNN (attention-LSTM over T=128 steps).

v2: tensor-parallel over the 4H gate dim across 8 cores, with a stacked
batch layout [128 partitions = 2 h-col-halves x 64 batch] so every
elementwise / ScalarE op runs on all 128 partitions, and every matmul is
col-tiled (tile_position) so both 64-partition output halves compute
concurrently.

Per-core layouts (core j owns h-cols [128j, 128j+128)):
  pa   [128, 256] f32   partition 64a+n = batch n, h-col half a; free =
                        4 gates x 64 cols (i|f|o|g)
  c    [128, 64]  f32   cell state, h_bf [128, 64] bf16 hidden
  hpay [128, 64]  bf16  exchange payload: hpay[64a+i, n] = h[n, 128j+64a+i]
  spay [128, 16]  bf16  score partials: cols 0:8 = own l-half (l = 8a+lh),
                        cols 8:16 = other half (uniform convention across
                        cores; downstream ops never need true l order)

Score partials are computed ON THE PE: one col-tiled all-pairs matmul
(hpay^T x scaled-A), then a mask+strided-reduce diag extraction, then a
permutation matmul (swap partition halves) to fill the other l-half.

Exchange: two AllGathers per step (h 16KB, scores 4KB) triggered
back-to-back so they overlap on the collective engine; single recv DMA per
payload on the Sync HWDGE queue.
"""
import numpy as np
import ml_dtypes

import bass_rust
import concourse.bass as bass
import concourse.mybir as mybir
from concourse import tile
from concourse.alu_op_type import AluOpType
from concourse.bass_utils import run_bass_kernel_spmd

BF16 = ml_dtypes.bfloat16
F32 = mybir.dt.float32
BF = mybir.dt.bfloat16
AF = mybir.ActivationFunctionType
AX = mybir.AxisListType

N, T, D, H, L, R = 64, 128, 512, 1024, 16, 8
HS, GS = H // R, 4 * H // R  # 128, 512
LH = L // 2  # 8
SCALE = 1.0 / np.sqrt(H)


def _split_waits(nc, cap=1):
    """Walrus rejects >cap sync waits per instruction; hoist extras onto
    preceding same-engine NOPs."""
    ctr = 0
    for fn in nc.m.functions:
        for bb in fn.blocks:
            out, changed = [], False
            for ins in bb.instructions:
                si = ins.sync_info
                if si is not None and si.on_wait and len(si.on_wait) > cap:
                    waits = list(si.on_wait)
                    extra, keep = waits[:-cap], waits[-cap:]
                    for i in range(0, len(extra), cap):
                        out.append(bass_rust.InstNoOp(
                            name=f"zz_waitsplit_{ctr}", engine=ins.engine,
                            sync_info=bass_rust.SyncInfo(
                                on_wait=extra[i:i + cap], on_update=[])))
                        ctr += 1
                    ins.sync_info = bass_rust.SyncInfo(
                        on_wait=keep, on_update=list(si.on_update or []))
                    changed = True
                out.append(ins)
            if changed:
                bb.instructions = out
    return ctr


def _prep_inputs(x, A, Wx, Wh, Wattn, b):
    x = np.asarray(x, np.float32)
    A_flat = np.asarray(A, np.float32).reshape(N, H, L)
    Wx = np.asarray(Wx, np.float32)
    Wh = np.asarray(Wh, np.float32)
    Wattn = np.asarray(Wattn, np.float32)
    b = np.asarray(b, np.float32)

    h0 = A_flat.mean(axis=2).astype(np.float32)          # (N, H)
    scores0 = (np.einsum('nh,nhl->nl', h0, A_flat) * SCALE)  # (N, L)

    # x transposed per step/k-tile: xT[t, kt, p, n] = x[n, t, 128kt+p]
    xT = np.ascontiguousarray(
        x.transpose(1, 2, 0).reshape(T, 4, 128, N)).astype(BF16)
    # asTf for the Bst precompute (raw A): [p, r, l, n']
    asTf = np.ascontiguousarray(
        A_flat.transpose(1, 2, 0).reshape(8, 128, L, N).transpose(1, 0, 2, 3)
    ).astype(BF16)
    # estack delta mask: dmE[64a+n', lh, n] = delta(n', n)
    eye = np.eye(N, dtype=np.float32)
    dmE = np.broadcast_to(
        np.concatenate([eye, eye], axis=0)[:, None, :], (128, LH, N))
    dmE = np.ascontiguousarray(dmE).astype(BF16)
    # all-pairs diag mask: dm[64a+n, n'*8+lh] = delta(n, n')
    dm = np.concatenate([eye, eye], axis=0)  # (128, 64)
    dm = np.ascontiguousarray(
        np.broadcast_to(dm[:, :, None], (128, N, LH)).reshape(128, N * LH)
    ).astype(BF16)
    # half-swap permutation: psw = eyesw^T @ x -> out[m] = x[m XOR 64]
    eyesw = np.zeros((128, 128), dtype=np.float32)
    for m in range(128):
        eyesw[m ^ 64, m] = 1.0
    eyesw = eyesw.astype(BF16)
    # identities for the two half transposes (both partition halves)
    eyes2 = np.concatenate([np.eye(N), np.eye(N)], axis=0).astype(BF16)

    in_maps = []
    for j in range(R):
        colsA = np.array([g * H + j * HS + i for g in range(4) for i in range(64)])
        colsB = colsA + 64
        cols = np.concatenate([colsA, colsB])  # 512
        # h0T slot layout (AllGather is rank-ordered): h0T[p, k, n] =
        # h0[n, k*128 + p]
        h0T = np.empty((128, R, N), dtype=np.float32)
        for k in range(R):
            h0T[:, k, :] = h0[:, k * HS:(k + 1) * HS].T
        # initial scores in payload convention
        s0 = np.empty((128, L), dtype=np.float32)
        for a in range(2):
            rows = slice(64 * a, 64 * a + 64)
            s0[rows, 0:LH] = scores0[:, 8 * a:8 * a + 8]
            s0[rows, LH:L] = scores0[:, 8 * (1 - a):8 * (1 - a) + 8]
        # c0 stacked
        c0 = np.empty((128, 64), dtype=np.float32)
        for a in range(2):
            c0[64 * a:64 * a + 64, :] = h0[:, j * HS + 64 * a:j * HS + 64 * a + 64]
        # whj[k, p, f]: rows Wh[k*128+p] (rank-ordered slots), gate-layout cols
        whj = Wh.reshape(R, HS, 4 * H)[:, :, cols]
        wxj = Wx.reshape(4, 128, 4 * H)[:, :, cols]
        waj = Wattn.reshape(8, 128, 4 * H)[:, :, cols]
        # alnP for the DVE score partials: alnP[64a+n, c, i] =
        #     SCALE * A[n, 128j+64a+i, l(a,c)], with the per-half column
        #     permutation l(a,c) = 8a+c (c<8, "own" l-half) else
        #     8(1-a)+(c-8) ("other" half) so downstream stays uniform.
        asl = A_flat[:, j * HS:(j + 1) * HS, :] * SCALE   # (n, 128, L)
        alnP = np.empty((128, L, 64), dtype=np.float32)
        for a in range(2):
            rows = slice(64 * a, 64 * a + 64)
            blk = asl[:, 64 * a:64 * a + 64, :]           # (n, 64 i, L)
            for c in range(L):
                l = 8 * a + c if c < LH else 8 * (1 - a) + (c - LH)
                alnP[rows, c, :] = blk[:, :, l]
        alnP = np.ascontiguousarray(alnP).astype(BF16)
        in_maps.append({
            "xT": xT,
            "whj": np.ascontiguousarray(whj).astype(BF16),
            "wxj": np.ascontiguousarray(wxj).astype(BF16),
            "waj": np.ascontiguousarray(waj).astype(BF16),
            "brep": np.tile(b[cols], (128, 1)).astype(np.float32),
            "asTf": asTf,
            "alnP": alnP,
            "dmE": dmE,
            "eyesw": eyesw,
            "eyes2": eyes2,
            "h0T": np.ascontiguousarray(h0T).astype(BF16),
            "s0": s0.astype(np.float32),
            "c0": c0,
        })
    return in_maps


def _build():
    nc = bass.Bass("TRN2", target_bir_lowering=False, debug=False, num_devices=R)
    rg = [list(range(R))]

    xT_d = nc.dram_tensor("xT", [T, 4, 128, N], BF, kind="ExternalInput")
    whj_d = nc.dram_tensor("whj", [R, 128, GS], BF, kind="ExternalInput")
    wxj_d = nc.dram_tensor("wxj", [4, 128, GS], BF, kind="ExternalInput")
    waj_d = nc.dram_tensor("waj", [R, 128, GS], BF, kind="ExternalInput")
    brep_d = nc.dram_tensor("brep", [128, GS], F32, kind="ExternalInput")
    asTf_d = nc.dram_tensor("asTf", [128, 8, L, N], BF, kind="ExternalInput")
    alnP_d = nc.dram_tensor("alnP", [128, L, 64], BF, kind="ExternalInput")
    dmE_d = nc.dram_tensor("dmE", [128, LH, N], BF, kind="ExternalInput")
    eyesw_d = nc.dram_tensor("eyesw", [128, 128], BF, kind="ExternalInput")
    eyes2_d = nc.dram_tensor("eyes2", [128, N], BF, kind="ExternalInput")
    h0T_d = nc.dram_tensor("h0T", [128, R, N], BF, kind="ExternalInput")
    s0_d = nc.dram_tensor("s0", [128, L], F32, kind="ExternalInput")
    c0_d = nc.dram_tensor("c0", [128, 64], F32, kind="ExternalInput")
    out_d = nc.dram_tensor("out", [128, T, 64], BF, kind="ExternalOutput")

    with tile.TileContext(nc) as tc:
        with tc.tile_pool(name="const", bufs=1) as cp, \
             tc.tile_pool(name="state", bufs=1) as st, \
             tc.tile_pool(name="dram", bufs=2, space="DRAM") as dp:

            whj = cp.tile([128, R, GS], BF, name="whj")
            wxj = cp.tile([128, 4, GS], BF, name="wxj")
            brep = cp.tile([128, GS], F32, name="brep")
            alnP = cp.tile([128, L, 64], BF, name="alnP")
            dmE = cp.tile([128, LH, N], BF, name="dmE")
            eyesw = cp.tile([128, 128], BF, name="eyesw")
            eyes2 = cp.tile([128, N], BF, name="eyes2")
            Bst = cp.tile([128, LH, GS], BF, name="Bst")
            nc.sync.dma_start(out=whj[:, :, :], in_=whj_d.rearrange("k p g -> p k g"))
            nc.sync.dma_start(out=wxj[:, :, :], in_=wxj_d.rearrange("k p g -> p k g"))
            nc.sync.dma_start(out=brep[:, :], in_=brep_d[:, :])
            nc.sync.dma_start(out=alnP[:, :, :], in_=alnP_d[:, :, :])
            nc.sync.dma_start(out=dmE[:, :, :], in_=dmE_d[:, :, :])
            nc.sync.dma_start(out=eyesw[:, :], in_=eyesw_d[:, :])
            nc.sync.dma_start(out=eyes2[:, :], in_=eyes2_d[:, :])

            c = st.tile([128, 64], F32, name="c")
            nc.sync.dma_start(out=c[:, :], in_=c0_d[:, :])

            # ---- precompute Bst[64a'+n', lh, :] = A_{8a'+lh}[n']^T @ Wattn_j + b
            with tc.tile_pool(name="pre", bufs=1) as pp, \
                 tc.tile_pool(name="ps_b", bufs=2, space="PSUM") as ps_b:
                asTf = pp.tile([128, 8, L, N], BF, name="asTf")
                waj = pp.tile([128, 8, GS], BF, name="waj")
                nc.sync.dma_start(out=asTf[:, :, :, :], in_=asTf_d[:, :, :, :])
                nc.sync.dma_start(out=waj[:, :, :], in_=waj_d.rearrange("k p g -> p k g"))
                for lh in range(LH):
                    pb = ps_b.tile([128, GS], F32, name="pb", tag="pb")
                    for r in range(8):
                        nc.tensor.matmul(pb[0:64, :], asTf[:, r, lh, :],
                                         waj[:, r, :], start=(r == 0),
                                         stop=(r == 7), tile_position=(0, 0))
                        nc.tensor.matmul(pb[64:128, :], asTf[:, r, 8 + lh, :],
                                         waj[:, r, :], start=(r == 0),
                                         stop=(r == 7), tile_position=(0, 64))
                    nc.vector.tensor_add(out=Bst[:, lh, :], in0=pb[:, :],
                                         in1=brep[:, :])

            with tc.tile_pool(name="wk", bufs=2) as wk, \
                 tc.tile_pool(name="ps_a", bufs=2, space="PSUM") as ps_a, \
                 tc.tile_pool(name="ps_t", bufs=2, space="PSUM") as ps_t, \
                 tc.tile_pool(name="ps_s", bufs=2, space="PSUM") as ps_s, \
                 tc.tile_pool(name="ps_w", bufs=2, space="PSUM") as ps_w:

                hkt = wk.tile([128, R, N], BF, name="hkt0", tag="hkt")
                nc.sync.dma_start(out=hkt[:, :, :], in_=h0T_d[:, :, :])
                sc16 = wk.tile([128, L], F32, name="sc160", tag="sc16")
                nc.sync.dma_start(out=sc16[:, :], in_=s0_d[:, :])
                xtile = wk.tile([128, 4, N], BF, name="xt0", tag="xtile")
                nc.scalar.dma_start(out=xtile[:, :, :],
                                    in_=xT_d[0].rearrange("k p n -> p k n"))
                sprecv = None

                for t in range(T):
                    # ---- gate preactivations: x part (prev AG window)
                    pa = ps_a.tile([128, GS // 2], F32, name="pa", tag="pa")
                    for kt in range(4):
                        nc.tensor.matmul(pa[0:64, :], xtile[:, kt, :],
                                         wxj[:, kt, 0:256],
                                         start=(kt == 0), stop=False,
                                         tile_position=(0, 0))
                        nc.tensor.matmul(pa[64:128, :], xtile[:, kt, :],
                                         wxj[:, kt, 256:512],
                                         start=(kt == 0), stop=False,
                                         tile_position=(0, 64))

                    # ---- softmax chain (sc16 = summed scores, payload order)
                    if t > 0:
                        sc16 = wk.tile([128, L], F32, name="sc16", tag="sc16")
                        nc.vector.reduce_sum(
                            out=sc16[:, :],
                            in_=sprecv.rearrange("p r l -> p l r"),
                            axis=AX.X)
                    e = wk.tile([128, L], F32, name="e", tag="e")
                    nc.scalar.activation(e[:, :], sc16[:, :], AF.Exp)
                    se = wk.tile([128, 1], F32, name="se", tag="se")
                    nc.vector.reduce_sum(out=se[:, :], in_=e[:, :], axis=AX.X)
                    rse = wk.tile([128, 1], F32, name="rse", tag="rse")
                    nc.vector.reciprocal(out=rse[:, :], in_=se[:, :])
                    wl = wk.tile([128, LH], BF, name="wl", tag="wl")
                    nc.vector.tensor_scalar(out=wl[:, :], in0=e[:, 0:LH],
                                            scalar1=rse[:, 0:1], scalar2=None,
                                            op0=AluOpType.mult)
                    estack = wk.tile([128, LH, N], BF, name="estack",
                                     tag="estack")
                    nc.vector.tensor_tensor(
                        out=estack[:, :, :], in0=dmE[:, :, :],
                        in1=wl[:, :, None].broadcast_to((128, LH, N)),
                        op=AluOpType.mult)

                    # ---- h part, then attn part
                    for r in range(8):
                        nc.tensor.matmul(pa[0:64, :], hkt[:, r, :],
                                         whj[:, r, 0:256], start=False,
                                         stop=False, tile_position=(0, 0))
                        nc.tensor.matmul(pa[64:128, :], hkt[:, r, :],
                                         whj[:, r, 256:512], start=False,
                                         stop=False, tile_position=(0, 64))
                    for lh in range(LH):
                        nc.tensor.matmul(pa[0:64, :], estack[:, lh, :],
                                         Bst[:, lh, 0:256], start=False,
                                         stop=(lh == LH - 1),
                                         tile_position=(0, 0))
                        nc.tensor.matmul(pa[64:128, :], estack[:, lh, :],
                                         Bst[:, lh, 256:512], start=False,
                                         stop=(lh == LH - 1),
                                         tile_position=(0, 64))

                    # ---- activations + cell update (gates i|f|o|g x 64)
                    th3 = wk.tile([128, 192], F32, name="th3", tag="th3")
                    nc.scalar.activation(th3[:, :], pa[:, 0:192], AF.Tanh,
                                         scale=0.5)
                    gt = wk.tile([128, 64], F32, name="gt", tag="gt")
                    nc.scalar.activation(gt[:, :], pa[:, 192:256], AF.Tanh)
                    sig = wk.tile([128, 192], F32, name="sig", tag="sig")
                    nc.vector.tensor_scalar(out=sig[:, :], in0=th3[:, :],
                                            scalar1=1.0, scalar2=0.5,
                                            op0=AluOpType.add,
                                            op1=AluOpType.mult)
                    t1 = wk.tile([128, 64], F32, name="t1", tag="t1")
                    nc.vector.tensor_mul(out=t1[:, :], in0=sig[:, 0:64],
                                         in1=gt[:, :])
                    nc.vector.tensor_mul(out=c[:, :], in0=sig[:, 64:128],
                                         in1=c[:, :])
                    nc.vector.tensor_add(out=c[:, :], in0=c[:, :], in1=t1[:, :])
                    tanc = wk.tile([128, 64], F32, name="tanc", tag="tanc")
                    nc.scalar.activation(tanc[:, :], c[:, :], AF.Tanh)
                    h_bf = wk.tile([128, 64], BF, name="h_bf", tag="h_bf")
                    nc.vector.tensor_mul(out=h_bf[:, :], in0=sig[:, 128:192],
                                         in1=tanc[:, :])
                    nc.sync.dma_start(out=out_d[:, t, :], in_=h_bf[:, :])
                    if t == T - 1:
                        break

                    # ---- h payload: two half transposes into one psum tile
                    pt = ps_t.tile([128, N], BF, name="pt", tag="pt")
                    nc.tensor.transpose(pt[0:64, :], h_bf[0:64, :],
                                        eyes2[0:64, :])
                    nc.tensor.transpose(pt[64:128, :], h_bf[64:128, :],
                                        eyes2[64:128, :])
                    # score partial half-sums on DVE (start right at h_bf):
                    # spt[64a+n, c, i] = alnP * h; spthalf = sum_i
                    spt = wk.tile([128, L, 64], BF, name="spt", tag="spt")
                    nc.vector.tensor_tensor(
                        out=spt[:, :, :], in0=alnP[:, :, :],
                        in1=h_bf[:, None, :].broadcast_to((128, L, 64)),
                        op=AluOpType.mult)
                    hpay = wk.tile([128, N], BF, name="hpay", tag="hpay")
                    nc.vector.tensor_copy(out=hpay[:, :], in_=pt[:, :])
                    sendHS = dp.tile([128 * (N + L)], BF, name="sendHS",
                                     tag="sendHS")
                    nc.scalar.dma_start(
                        out=sendHS[0:128 * N].rearrange("(p n) -> p n", p=128),
                        in_=hpay[:, :])
                    spthalf = wk.tile([128, L], BF, name="spthalf",
                                      tag="spthalf")
                    with nc.allow_low_precision(reason="bf16 score partials"):
                        nc.vector.reduce_sum(out=spthalf[:, :],
                                             in_=spt[:, :, :], axis=AX.X)
                    # other-half contribution: swap partition halves, then
                    # column-rotate by 8 during the adds
                    psw = ps_w.tile([128, L], F32, name="psw", tag="psw")
                    nc.tensor.matmul(psw[:, :], eyesw[:, :], spthalf[:, :],
                                     start=True, stop=True)
                    spay = wk.tile([128, L], BF, name="spay", tag="spay")
                    nc.vector.tensor_add(out=spay[:, 0:LH],
                                         in0=spthalf[:, 0:LH],
                                         in1=psw[:, LH:L])
                    nc.vector.tensor_add(out=spay[:, LH:L],
                                         in0=spthalf[:, LH:L],
                                         in1=psw[:, 0:LH])
                    nc.scalar.dma_start(
                        out=sendHS[128 * N:].rearrange("(p l) -> p l", p=128),
                        in_=spay[:, :])
                    recvHS = dp.tile([R, 128 * (N + L)], BF, name="recvHS",
                                     tag="recvHS", addr_space="Shared")
                    nc.gpsimd.collective_compute(
                        "AllGather", AluOpType.bypass, replica_groups=rg,
                        ins=[sendHS[:].opt()], outs=[recvHS[:, :].opt()])

                    # ---- receives (scores on sync, h on scalar) + x prefetch
                    sprecv = wk.tile([128, R, L], BF, name="sprecv",
                                     tag="sprecv")
                    nc.sync.dma_start(
                        out=sprecv[:, :, :],
                        in_=recvHS[:, 128 * N:].rearrange("r (p l) -> p r l",
                                                          p=128))
                    hkt = wk.tile([128, R, N], BF, name="hkt", tag="hkt")
                    nc.scalar.dma_start(
                        out=hkt[:, :, :],
                        in_=recvHS[:, 0:128 * N].rearrange("r (p n) -> p r n",
                                                           p=128))
                    xtile = wk.tile([128, 4, N], BF, name="xt", tag="xtile")
                    nc.sync.dma_start(
                        out=xtile[:, :, :],
                        in_=xT_d[t + 1].rearrange("k p n -> p k n"))

    _split_waits(nc, cap=1)
    return nc


_NC_CACHE = None


def _assemble(res) -> np.ndarray:
    out = np.zeros((N, T, H), dtype=np.float32)
    for j, r in enumerate(res.results):
        o = np.asarray(r["out"]).astype(np.float32)  # [128, T, 64]
        o = o.reshape(2, 64, T, 64)                  # [a, n, t, i]
        out[:, :, j * HS:j * HS + 64] = o[0]
        out[:, :, j * HS + 64:j * HS + 128] = o[1]
    return out


def kernel(**inputs) -> np.ndarray:
    global _NC_CACHE
    in_maps = _prep_inputs(**inputs)
    if _NC_CACHE is None:
        _NC_CACHE = _build()
    res = run_bass_kernel_spmd(_NC_CACHE, in_maps, core_ids=list(range(R)))
    return _assemble(res)
